# revision 7
# baseline (speedup 1.0000x reference)
"""DeepSeekMoE on 8 Trainium2 NeuronCores.

Strategy
--------
Routing (tiny: [2048,1536]@[1536,6]) is computed on host with jax-on-CPU,
mirroring the reference bit-for-bit, so the top-2 expert selection cannot
flip. Because the Bass kernel is compiled *after* the routing is known, all
token counts are compile-time constants — no dynamic control flow on device.

Tokens are gathered into per-expert column segments of a transposed
activation matrix XT [H, C] (C = 2048 shared cols + 4096 routed pair cols).
Every core runs the identical grouped-GEMM program on a 512-wide slice of
the intermediate dimension (tensor-parallel over I): for each expert
segment, out^T += Wd_sl^T @ (w * silu(Wg_sl^T x) * (Wu_sl^T x)).
This layout needs zero on-device transposes: gate/up use Wg/Wu directly as
the stationary operand and XT tiles as the moving operand; down uses Wd
directly with the gated activation already I-major in SBUF.

The two shared experts form one segment whose down-projection accumulates
both experts in PSUM (Wd pre-scaled by 1/2 on host); routed segments apply
the per-token combine weight to the gated activation before the down
matmul. The 8 per-core partial outputs are summed on host, and the routed
pair columns are gathered back per token (pure fancy indexing, no
scatter-add). Compute is in bf16 with fp32 PSUM accumulation; sparse FLOPs
only (top-2 of 6 routed experts), ~39 GFLOP/core, perfectly balanced.
"""

import os
import sys

if "/opt/trn_rl_repo" not in sys.path:
    sys.path.insert(0, "/opt/trn_rl_repo")

import numpy as np
import ml_dtypes

import concourse.bass as bass
import concourse.mybir as mybir
import concourse.tile as tile
from concourse import bacc
from concourse.bass_utils import run_bass_kernel_spmd

H = 1536
I = 4096
T = 2048
E_SH = 2
E_RT = 6
E = E_SH + E_RT  # expert slots: [s0, s1, r0..r5]
TOP_K = 2
N_CORES = 8
ISL = I // N_CORES  # 512 per-core slice of the intermediate dim
KT = H // 128  # 12 contraction tiles for gate/up
MT = ISL // 128  # 4 partition tiles of the I-slice
JT = H // 128  # 12 output H tiles for down
NB = 512  # token-column block (one PSUM bank of fp32)
BF16 = mybir.dt.bfloat16
F32 = mybir.dt.float32

# Stashed by kernel() for the test harness (exec_time_ns when BASS_TRACE=1).
LAST_RESULT = None


def _route(xf, Wr, rb):
    """Top-2 routing on host, on jax-CPU with the reference's exact ops."""
    import jax

    cpu = jax.devices("cpu")[0]
    xj = jax.device_put(xf, cpu)
    wj = jax.device_put(np.asarray(Wr, np.float32), cpu)
    rj = jax.device_put(np.asarray(rb, np.float32), cpu)
    logits = xj @ wj + rj
    probs = jax.nn.softmax(logits, axis=-1)
    scores, idx = jax.lax.top_k(probs, TOP_K)
    scores = scores / scores.sum(axis=-1, keepdims=True)
    return np.asarray(idx), np.asarray(scores)


_NC_CACHE = {}


def _build_nc(seg_key):
    """seg_key: tuple of routed-expert token counts (n_0..n_5)."""
    if seg_key in _NC_CACHE:
        return _NC_CACHE[seg_key]

    C = T + sum(seg_key)
    segs = [((0, 1), 0, T)]
    off = T
    for e, n in enumerate(seg_key):
        segs.append(((E_SH + e,), off, n))
        off += n

    nc = bacc.Bacc(None, target_bir_lowering=False, debug=False)
    XT = nc.declare_dram_parameter("XT", [H, C], BF16, isOutput=False)
    WBp = nc.declare_dram_parameter("WB", [128, C], BF16, isOutput=False)
    WG = nc.declare_dram_parameter("WG", [E, H, ISL], BF16, isOutput=False)
    WU = nc.declare_dram_parameter("WU", [E, H, ISL], BF16, isOutput=False)
    WD = nc.declare_dram_parameter("WD", [E, ISL, H], BF16, isOutput=False)
    OUT = nc.declare_dram_parameter("OUT", [H, C], F32, isOutput=True)

    XT_r = XT.rearrange("(k p) c -> p k c", p=128)
    OUT_r = OUT.rearrange("(j p) c -> p j c", p=128)

    silu = mybir.ActivationFunctionType.Silu

    with tile.TileContext(nc) as tc:
        with (
            tc.tile_pool(name="wpool", bufs=3) as wpool,
            tc.tile_pool(name="xpool", bufs=3) as xpool,
            tc.tile_pool(name="hpool", bufs=4) as hpool,
            tc.tile_pool(name="hwpool", bufs=24) as hwpool,
            tc.tile_pool(name="opool", bufs=4) as opool,
            tc.tile_pool(name="gupool", bufs=4, space="PSUM") as gupool,
            tc.tile_pool(name="dnpool", bufs=3, space="PSUM") as dnpool,
        ):
            # Deferred down-projection: emit block b's down matmuls after
            # block b+1's gate/up, so the PE never stalls on the ACT/DVE
            # chain that produces the gated activations, and segment
            # transitions (weight DMAs) are covered by the previous
            # block's down work.
            pending = []

            def emit_down(state):
                wts_, hw_tiles_, cb_, nb_ = state
                for j in range(JT):
                    pd = dnpool.tile([128, nb_], F32, tag="dn", name="pd")
                    last_i = len(hw_tiles_) - 1
                    for i, (es, km, hwt) in enumerate(hw_tiles_):
                        nc.tensor.matmul(
                            pd[:],
                            wts_[es][2][:, km, j * 128 : (j + 1) * 128],
                            hwt[:],
                            start=(i == 0),
                            stop=(i == last_i),
                        )
                    ot = opool.tile([128, nb_], F32, tag="o", name="ot")
                    nc.vector.tensor_copy(ot[:], pd[:])
                    nc.sync.dma_start(OUT_r[:, j, cb_ : cb_ + nb_], ot[:])

            for slots, c0, n in segs:
                if n == 0:
                    continue
                # Weight loads, k-chunked so the first matmul only waits for
                # one 128-row slice, issued on the ACT HWDGE ring to stay
                # out of the SP ring's xt/out FIFO.
                wts = {}
                for es in slots:
                    wg = wpool.tile([128, KT, ISL], BF16, tag="wg", name=f"wg{es}")
                    wu = wpool.tile([128, KT, ISL], BF16, tag="wu", name=f"wu{es}")
                    wgr = WG[es].rearrange("(k p) m -> p k m", p=128)
                    wur = WU[es].rearrange("(k p) m -> p k m", p=128)
                    for k in range(KT):
                        nc.scalar.dma_start(wg[:, k, :], wgr[:, k, :])
                    for k in range(KT):
                        nc.scalar.dma_start(wu[:, k, :], wur[:, k, :])
                    wd = wpool.tile([128, MT, H], BF16, tag="wd", name=f"wd{es}")
                    wdr = WD[es].rearrange("(km p) h -> p km h", p=128)
                    for km in range(MT):
                        nc.scalar.dma_start(wd[:, km, :], wdr[:, km, :])
                    wts[es] = (wg, wu, wd)
                is_shared = len(slots) > 1

                # Evenly split the segment into blocks <= NB (a lone small
                # tail block would run at the LDWEIGHTS-floor issue rate).
                nblk = -(-n // NB)
                bounds = [c0 + (n * i) // nblk for i in range(nblk + 1)]
                for bi in range(nblk):
                    cb = bounds[bi]
                    nb = bounds[bi + 1] - cb
                    xt = xpool.tile([128, KT, nb], BF16, tag="xt", name="xt")
                    nc.sync.dma_start(xt[:], XT_r[:, :, cb : cb + nb])
                    if not is_shared:
                        wb = xpool.tile([128, nb], BF16, tag="wb", name="wb")
                        nc.sync.dma_start(wb[:], WBp[:, cb : cb + nb])

                    hw_tiles = []
                    for es in slots:
                        wg, wu, wd = wts[es]
                        for m in range(MT):
                            pg = gupool.tile([128, nb], F32, tag="gu", name="pg")
                            for k in range(KT):
                                nc.tensor.matmul(
                                    pg[:],
                                    wg[:, k, m * 128 : (m + 1) * 128],
                                    xt[:, k, :],
                                    start=(k == 0),
                                    stop=(k == KT - 1),
                                )
                            hg = hpool.tile([128, nb], BF16, tag="hg", name="hg")
                            nc.scalar.activation(hg[:], pg[:], silu)
                            pu = gupool.tile([128, nb], F32, tag="gu", name="pu")
                            for k in range(KT):
                                nc.tensor.matmul(
                                    pu[:],
                                    wu[:, k, m * 128 : (m + 1) * 128],
                                    xt[:, k, :],
                                    start=(k == 0),
                                    stop=(k == KT - 1),
                                )
                            us = hpool.tile([128, nb], BF16, tag="us", name="us")
                            nc.scalar.copy(us[:], pu[:])
                            hwt = hwpool.tile([128, nb], BF16, tag="hw", name="hw")
                            nc.vector.tensor_mul(hwt[:], hg[:], us[:])
                            if not is_shared:
                                nc.vector.tensor_mul(hwt[:], hwt[:], wb[:])
                            hw_tiles.append((es, m, hwt))

                    if pending:
                        emit_down(pending.pop())
                    pending.append((wts, hw_tiles, cb, nb))

            while pending:
                emit_down(pending.pop())

    nc.compile()
    _NC_CACHE[seg_key] = nc
    return nc


def kernel(x, Wg_s, Wu_s, Wd_s, Wg_r, Wu_r, Wd_r, Wr, rb):
    global LAST_RESULT
    xf = np.ascontiguousarray(np.asarray(x, np.float32).reshape(T, H))
    idx, sc = _route(xf, Wr, rb)

    # Per-expert token lists (compile-time constants for this call).
    tok_lists = []
    for e in range(E_RT):
        hit = idx == e  # [T, K]
        tok_lists.append(np.nonzero(hit.any(axis=1))[0])
    seg_key = tuple(len(t) for t in tok_lists)
    C = T + sum(seg_key)

    # Host-side gather into the column space.
    xfT_bf = np.ascontiguousarray(xf.T).astype(ml_dtypes.bfloat16)
    XTc = np.empty((H, C), dtype=ml_dtypes.bfloat16)
    XTc[:, :T] = xfT_bf
    wcol = np.ones((C,), np.float32)
    col_of = np.zeros((T, TOP_K), np.int64)
    off = T
    for e in range(E_RT):
        toks = tok_lists[e]
        n = len(toks)
        if n:
            XTc[:, off : off + n] = xfT_bf[:, toks]
            kk = np.where(idx[toks, 0] == e, 0, 1)
            wcol[off : off + n] = sc[toks, kk]
            col_of[toks, kk] = np.arange(off, off + n)
        off += n
    WBm = np.ascontiguousarray(
        np.broadcast_to(wcol.astype(ml_dtypes.bfloat16)[None, :], (128, C))
    )

    # Expert-slot weight stacks (shared first, down pre-scaled by 1/E_SH),
    # sliced per core along the intermediate dim.
    wg_bf = np.concatenate(
        [np.asarray(Wg_s, np.float32), np.asarray(Wg_r, np.float32)], axis=0
    ).astype(ml_dtypes.bfloat16)
    wu_bf = np.concatenate(
        [np.asarray(Wu_s, np.float32), np.asarray(Wu_r, np.float32)], axis=0
    ).astype(ml_dtypes.bfloat16)
    wd_bf = np.concatenate(
        [np.asarray(Wd_s, np.float32) / E_SH, np.asarray(Wd_r, np.float32)], axis=0
    ).astype(ml_dtypes.bfloat16)

    in_maps = []
    for c in range(N_CORES):
        sl = slice(c * ISL, (c + 1) * ISL)
        in_maps.append(
            {
                "XT": XTc,
                "WB": WBm,
                "WG": np.ascontiguousarray(wg_bf[:, :, sl]),
                "WU": np.ascontiguousarray(wu_bf[:, :, sl]),
                "WD": np.ascontiguousarray(wd_bf[:, sl, :]),
            }
        )

    nc = _build_nc(seg_key)
    res = run_bass_kernel_spmd(nc, in_maps, core_ids=list(range(N_CORES)))
    LAST_RESULT = res

    osum = res.results[0]["OUT"].astype(np.float32, copy=True)
    for c in range(1, N_CORES):
        osum += res.results[c]["OUT"]

    outT = osum[:, :T] + osum[:, col_of[:, 0]] + osum[:, col_of[:, 1]]
    return np.ascontiguousarray(outT.T).reshape(1, T, H).astype(np.float32)
